# revision 1
# baseline (speedup 1.0000x reference)
"""PhysicsGuidedLoss TRN2 kernel — engine-split design.

Same edge-sharding and gather layout as the baseline kernel, but:
  - table rows in bf16 (256B) when GATHER_BF16, halving gather bytes
  - DVE does only the 3 coefficient multiplies (bf16, 2 elem/cycle/lane)
  - PE accumulates r = pd - a0 - a1 - b into PSUM via +/- identity matmuls
  - ACT (scalar engine) computes Square with accum_out, fusing square+reduce
  - data loss: DVE subtract + ACT square/accum

"""
import os
import sys

import numpy as np

if "/opt/trn_rl_repo" not in sys.path:
    sys.path.insert(0, "/opt/trn_rl_repo")

import ml_dtypes
import concourse.bass as bass
import concourse.mybir as mybir
import concourse.tile as tile
from concourse import bacc
from concourse.bass_utils import run_bass_kernel_spmd

P = 128
B, N, E = 64, 10000, 320000
LAMBDA_PHY = 0.3
NCORES = 8
EPC = E // NCORES            # 40000 edges per core
NIDX = 1024                  # idxs per gather (HW limit)
SLOTS = NIDX // P            # slot groups per gather chunk
NCHUNK = (EPC + NIDX - 1) // NIDX  # gather chunks per core
EPAD = NCHUNK * NIDX         # padded edges per core
NDL = N // NCORES            # 1250 data-loss columns per core
DL_F = B * NDL // P          # 625

USE_BF16 = True
USE_PE = True
USE_ACT = True

FP = mybir.dt.float32
BF = mybir.dt.bfloat16
I16 = mybir.dt.int16
GD = BF if USE_BF16 else FP  # gather/table dtype

LAST_EXEC_NS = None
LAST_PROFILE = None

_NC_CACHE = {}


def _build_nc():
    key = (USE_BF16, USE_PE, USE_ACT, NIDX)
    if key in _NC_CACHE:
        return _NC_CACHE[key]
    nc = bacc.Bacc(None, target_bir_lowering=False, num_swdge_queues=4)

    table_d = nc.declare_dram_parameter("table", [N, P], GD, isOutput=False)
    sidx_d = nc.declare_dram_parameter("sidx", [P, EPAD // 16], I16, isOutput=False)
    didx_d = nc.declare_dram_parameter("didx", [P, EPAD // 16], I16, isOutput=False)
    c0_d = nc.declare_dram_parameter("c0a", [P, EPAD // P], GD, isOutput=False)
    c1_d = nc.declare_dram_parameter("c1a", [P, EPAD // P], GD, isOutput=False)
    c2_d = nc.declare_dram_parameter("c2a", [P, EPAD // P], GD, isOutput=False)
    pdl_d = nc.declare_dram_parameter("pdl", [P, DL_F], FP, isOutput=False)
    tdl_d = nc.declare_dram_parameter("tdl", [P, DL_F], FP, isOutput=False)
    out_d = nc.declare_dram_parameter("partials", [P, 2], FP, isOutput=True)
    if USE_PE:
        ipos_d = nc.declare_dram_parameter("ipos", [P, P], GD, isOutput=False)
        ineg_d = nc.declare_dram_parameter("ineg", [P, P], GD, isOutput=False)

    G = EPAD // P  # total slot groups per core

    with tile.TileContext(nc) as tc:
        with tc.tile_pool(name="sbuf", bufs=1) as pool, \
             tc.tile_pool(name="psum", bufs=1, space="PSUM") as psum:
            sidx_t = pool.tile([P, EPAD // 16], I16)
            didx_t = pool.tile([P, EPAD // 16], I16)
            c0_t = pool.tile([P, G], GD)
            c1_t = pool.tile([P, G], GD)
            c2_t = pool.tile([P, G], GD)
            pdl_t = pool.tile([P, DL_F], FP)
            tdl_t = pool.tile([P, DL_F], FP)
            dd_t = pool.tile([P, DL_F], FP)
            phy_acc = pool.tile([P, 1], FP)
            dacc = pool.tile([P, 1], FP)
            chunk_accs = pool.tile([P, NCHUNK], FP)

            NBUF = 5
            gs_t = [pool.tile([P, SLOTS, P], GD, name=f"gs{i}") for i in range(NBUF)]
            gd_t = [pool.tile([P, SLOTS, P], GD, name=f"gd{i}") for i in range(NBUF)]
            a0_t = [pool.tile([P, SLOTS, B], GD, name=f"a0_{i}") for i in range(3)]
            a1_t = [pool.tile([P, SLOTS, B], GD, name=f"a1_{i}") for i in range(3)]
            b_t = [pool.tile([P, SLOTS, B], GD, name=f"b_{i}") for i in range(3)]
            sq_scr_t = [pool.tile([P, SLOTS * B], GD, name=f"sqs{i}") for i in range(2)]
            dl_scr = pool.tile([P, DL_F], FP)

            c_t = [pool.tile([P, SLOTS, B], GD, name=f"c_{i}") for i in range(3)]
            if USE_PE:
                ipos_t = pool.tile([P, P], GD)
                ineg_t = pool.tile([P, P], GD)
                ps_r = [psum.tile([P, SLOTS * B], FP, name=f"psr{i}") for i in range(3)]
            else:
                r_t = [pool.tile([P, SLOTS, B], GD, name=f"r_{i}") for i in range(2)]

            # split idx loads so the first gathers unblock before the
            # whole 1.3MB index stream lands (subtile deps): piece 1
            # covers the first 2 chunks, then the remainder; big data-loss
            # inputs stream last.
            icut = 2 * (NIDX // 16)
            nc.sync.dma_start(out=sidx_t[:, 0:icut], in_=sidx_d[:, 0:icut])
            nc.sync.dma_start(out=didx_t[:, 0:icut], in_=didx_d[:, 0:icut])
            nc.sync.dma_start(out=c0_t[:], in_=c0_d[:])
            nc.sync.dma_start(out=c1_t[:], in_=c1_d[:])
            nc.sync.dma_start(out=c2_t[:], in_=c2_d[:])
            nc.sync.dma_start(out=sidx_t[:, icut:], in_=sidx_d[:, icut:])
            nc.sync.dma_start(out=didx_t[:, icut:], in_=didx_d[:, icut:])
            nc.sync.dma_start(out=pdl_t[:], in_=pdl_d[:])
            nc.sync.dma_start(out=tdl_t[:], in_=tdl_d[:])

            mul = mybir.AluOpType.mult
            sub = mybir.AluOpType.subtract
            add = mybir.AluOpType.add

            if USE_PE:
                nc.sync.dma_start(out=ipos_t[:], in_=ipos_d[:])
                nc.sync.dma_start(out=ineg_t[:], in_=ineg_d[:])

            # data loss on DVE sub + ACT square/accum (or DVE fallback)
            nc.vector.tensor_tensor(out=dd_t[:], in0=pdl_t[:], in1=tdl_t[:], op=sub)
            if USE_ACT:
                nc.scalar.activation(out=dl_scr[:], in_=dd_t[:],
                                     func=mybir.ActivationFunctionType.Square,
                                     accum_out=dacc[:])
            else:
                nc.vector.tensor_tensor(out=dl_scr[:], in0=dd_t[:], in1=dd_t[:], op=mul)
                nc.vector.tensor_reduce(out=dacc[:], in_=dl_scr[:],
                                        axis=mybir.AxisListType.X, op=add)

            for j in range(NCHUNK):
                gs = gs_t[j % NBUF]
                gdt = gd_t[j % NBUF]
                so = j * SLOTS
                n_idx = NIDX
                n_real = min(EPC - j * NIDX, NIDX)
                col0 = j * (NIDX // 16)
                ncol = NIDX // 16

                if n_real < n_idx:
                    # pad edges: zero whole gather tiles (coeffs are 0 there too)
                    nc.vector.memset(gs[:], 0.0)
                    nc.vector.memset(gdt[:], 0.0)

                nc.gpsimd.dma_gather(
                    out_ap=gs[:, :, :], in_ap=table_d[:, :],
                    idxs_ap=sidx_t[:, col0:col0 + ncol],
                    num_idxs=n_idx, num_idxs_reg=n_real,
                    elem_size=P, queue_num=(2 * j) % 4)
                nc.gpsimd.dma_gather(
                    out_ap=gdt[:, :, :], in_ap=table_d[:, :],
                    idxs_ap=didx_t[:, col0:col0 + ncol],
                    num_idxs=n_idx, num_idxs_reg=n_real,
                    elem_size=P, queue_num=(2 * j + 1) % 4)

                c0b = c0_t[:, so:so + SLOTS, None].to_broadcast([P, SLOTS, B])
                c1b = c1_t[:, so:so + SLOTS, None].to_broadcast([P, SLOTS, B])
                c2b = c2_t[:, so:so + SLOTS, None].to_broadcast([P, SLOTS, B])

                a0 = a0_t[j % 3]
                a1 = a1_t[j % 3]
                bb = b_t[j % 3]
                cc = c_t[j % 3]
                nc.vector.tensor_tensor(out=a0[:], in0=gs[:, :, 0:B], in1=c0b, op=mul)
                nc.vector.tensor_tensor(out=a1[:], in0=gs[:, :, B:P], in1=c1b, op=mul)
                nc.vector.tensor_tensor(out=bb[:], in0=gdt[:, :, B:P], in1=c2b, op=mul)
                # c = pd - b (also folds the strided pd slice into one DVE op)
                nc.vector.tensor_tensor(out=cc[:], in0=gdt[:, :, 0:B], in1=bb[:], op=sub)

                if USE_PE:
                    ps = ps_r[j % 3]
                    nc.tensor.matmul(ps[:], ipos_t[:], cc[:], start=True, stop=False)
                    nc.tensor.matmul(ps[:], ineg_t[:], a0[:], start=False, stop=False)
                    nc.tensor.matmul(ps[:], ineg_t[:], a1[:], start=False, stop=True)
                    rsrc = ps[:]
                else:
                    r = r_t[j % 2 if False else 0]
                    nc.vector.tensor_tensor(out=cc[:], in0=cc[:], in1=a0[:], op=sub)
                    nc.vector.tensor_tensor(out=r[:], in0=cc[:], in1=a1[:], op=sub)
                    rsrc = r[:]

                if USE_ACT:
                    nc.scalar.activation(
                        out=sq_scr_t[j % 2][:], in_=rsrc,
                        func=mybir.ActivationFunctionType.Square,
                        accum_out=chunk_accs[:, j:j + 1])
                else:
                    sq = a0_t[(j + 1) % 3]  # reuse
                    nc.vector.tensor_tensor(out=sq[:], in0=rsrc, in1=rsrc, op=mul)
                    nc.vector.tensor_reduce(out=chunk_accs[:, j:j + 1], in_=sq[:],
                                            axis=mybir.AxisListType.XY, op=add)

            nc.vector.tensor_reduce(out=phy_acc[:], in_=chunk_accs[:],
                                    axis=mybir.AxisListType.X, op=add)
            nc.sync.dma_start(out=out_d[:, 0:1], in_=phy_acc[:])
            nc.sync.dma_start(out=out_d[:, 1:2], in_=dacc[:])

    nc.finalize()
    _NC_CACHE[key] = nc
    return nc


def _wrap_idx(idx_pad: np.ndarray) -> np.ndarray:
    w16 = idx_pad.reshape(EPAD // 16, 16).T
    return np.ascontiguousarray(np.tile(w16, (8, 1)))


def _arrange_coeff(c_shard: np.ndarray, np_gd) -> np.ndarray:
    cp = np.zeros(EPAD, np.float32)
    cp[:EPC] = c_shard
    return np.ascontiguousarray(cp.reshape(EPAD // P, P).T).astype(np_gd)


def kernel(**inputs) -> np.ndarray:
    global LAST_EXEC_NS, LAST_PROFILE
    pred = np.ascontiguousarray(np.asarray(inputs["pred"], dtype=np.float32))
    target = np.ascontiguousarray(np.asarray(inputs["target"], dtype=np.float32))
    prev_target = np.ascontiguousarray(np.asarray(inputs["prev_target"], dtype=np.float32))
    c0 = np.asarray(inputs["c0"], dtype=np.float32)
    c1 = np.asarray(inputs["c1"], dtype=np.float32)
    c2 = np.asarray(inputs["c2"], dtype=np.float32)
    edge_index = np.asarray(inputs["edge_index"])
    src = edge_index[0].astype(np.int16)
    dst = edge_index[1].astype(np.int16)

    np_gd = ml_dtypes.bfloat16 if USE_BF16 else np.float32
    table = np.ascontiguousarray(
        np.concatenate([pred.T, prev_target.T], axis=1)).astype(np_gd)
    ipos = np.eye(P, dtype=np_gd)
    ineg = (-np.eye(P)).astype(np_gd)

    in_maps = []
    for c in range(NCORES):
        esl = slice(c * EPC, (c + 1) * EPC)
        s_pad = np.full(EPAD, -1, np.int16)
        s_pad[:EPC] = src[esl]
        d_pad = np.full(EPAD, -1, np.int16)
        d_pad[:EPC] = dst[esl]
        nsl = slice(c * NDL, (c + 1) * NDL)
        in_maps.append({
            "table": table,
            "sidx": _wrap_idx(s_pad),
            "didx": _wrap_idx(d_pad),
            "c0a": _arrange_coeff(c0[esl], np_gd),
            "c1a": _arrange_coeff(c1[esl], np_gd),
            "c2a": _arrange_coeff(c2[esl], np_gd),
            "pdl": np.ascontiguousarray(pred[:, nsl].reshape(P, DL_F)),
            "tdl": np.ascontiguousarray(target[:, nsl].reshape(P, DL_F)),
        })
        if USE_PE:
            in_maps[-1]["ipos"] = ipos
            in_maps[-1]["ineg"] = ineg

    nc = _build_nc()
    res = run_bass_kernel_spmd(nc, in_maps, list(range(NCORES)))
    LAST_EXEC_NS = res.exec_time_ns
    LAST_PROFILE = res.profile_json

    phy_sum = 0.0
    data_sum = 0.0
    for c in range(NCORES):
        part = np.asarray(res.results[c]["partials"], dtype=np.float64)
        phy_sum += part[:, 0].sum()
        data_sum += part[:, 1].sum()

    data_loss = data_sum / (B * N)
    phy_loss = phy_sum / (B * E)
    total = data_loss + LAMBDA_PHY * phy_loss
    return np.array([total, data_loss, phy_loss], dtype=np.float32)

